# revision 33
# baseline (speedup 1.0000x reference)
"""Trainium2 Bass kernel for AttnPainterOil-style top-K stroke compositing.

Problem semantics (per pixel, fully independent):
  draw[n] = (n+1) * (alpha[n] > 0.1); top-K=10 of draw over N=256 strokes
  (descending) == the 10 highest-index strokes with alpha > 0.1.  Gather
  alpha/color at those indices and composite back-to-front over white.

Device formulation (front-to-back, strokes in descending index order):
maintain per-pixel transmittance T (init 1) and a raw pass-count R.  For
stroke s with host-masked alpha ae_s (= a * 1{a > 0.1}, fp32-exact mask
applied on host, shipped as fp16):
  gate m_s = 1{R_s < 10} with R_s = #passing among strokes < s (RAW count,
  independent of gating -- so it batches), ta_s = m_s * b_s * T_quad, where
  b_s are quad-local exclusive-prefix weights b_j = ae_j * prod_{i<j}(1-ae_i)
  (batch-precomputed).  Within a quad the gate mask is a suffix cut, so
  ta_j = m_j * b_j * T is exact and T_new = T - sum_j ta_j.

Scheduling facts measured on TRN2 for this kernel family:
  - DVE op = ~150ns dispatch + free_elems cycles; fp16 unit-stride runs 2x
    (STT runs 1x).  Broadcasts on outer dims keep 2x.
  - The ACT (scalar) queue is a serial second critical path: anything the
    serial chain needs (q for the first quads, b pos-0) must NOT ride it.
    ACT keeps only late-needed work: q for strokes 8+, pass bits, partial
    copies.
  - Each dma_start costs ~700ns issue on its queue engine; alpha goes first
    in small chunks, colors follow per-quad on the same Sync queue
    (concurrent big transfers on another queue starve the alpha DMA).
  - fp16 everywhere passes easily (measured 1.0e-3 vs 2e-2 tolerance).
  - Depth 20 suffices for this input (every pixel's 10th passing stroke is
    within the top 20; checked on host, exact numpy fallback otherwise).
  - PE accumulates ta*c into PSUM via fp16 identity matmuls.

Sharding: pure data parallel, one batch element per NeuronCore (B=8).
"""

import numpy as np

B, N, W, K = 8, 256, 128, 10
ALPHA_THRESH = 0.1
D = 20          # strokes processed from the top (host-verified sufficient)
P = 128         # partitions (pixel rows)
F = 128         # free dim (pixel cols)
NCORES = 8

_nc_cache = {}


def _build_nc(depth):
    import concourse.bass as bass  # noqa: F401
    import concourse.tile as tile
    from concourse import bacc, mybir
    from concourse.vector_clock import ScopedClock

    op = mybir.AluOpType
    act = mybir.ActivationFunctionType
    f16 = mybir.dt.float16
    f32 = mybir.dt.float32

    assert depth % 4 == 0
    NQ = depth // 4          # quads (5)
    NG = NQ - 2              # gated quads (strokes 8..depth-1)
    S2 = depth - 8           # strokes in the ACT-side alpha chunk

    class _OneShotTileContext(tile.TileContext):
        """TileContext with a slim exit: the drain alone (it waits on the
        global clock, including output-DMA completion) -- no all-engine
        barriers and no per-semaphore clears."""

        def _drain_and_barrier(self, tick_clock, wait_clock):
            drain_inst = self.nc.sync.drain()
            wait_clock.add_sem_waits(
                drain_inst.ins, ScopedClock({None: tick_clock.global_clock})
            )
            popped = self.nc._tile_sem_poison_stack.pop()
            assert popped is self._sem_poison

    nc = bacc.Bacc("TRN2", target_bir_lowering=False, debug=False)

    ae_d = nc.dram_tensor("ae_in", [P, depth * F], f16, kind="ExternalInput").ap()
    c_d = nc.dram_tensor("color_in", [P, depth * 3 * F], f16, kind="ExternalInput").ap()
    ident_d = nc.dram_tensor("ident_in", [P, P], f16, kind="ExternalInput").ap()
    out_d = nc.dram_tensor("out", [P, 3 * F], f16, kind="ExternalOutput").ap()

    with _OneShotTileContext(nc) as tc:
        with (
            tc.tile_pool(name="const", bufs=1) as constp,
            tc.tile_pool(name="slab", bufs=1) as slabp,
            tc.tile_pool(name="work", bufs=2) as workp,
            tc.tile_pool(name="prodp", bufs=2) as prodp,
            tc.tile_pool(name="psum", bufs=1, space="PSUM") as psump,
        ):
            # ident via SWDGE (gpsimd queue) so it doesn't delay the HWDGE
            # input stream; it's only needed by the first matmul.
            ident = constp.tile([P, P], f16)
            nc.gpsimd.dma_start(ident[:], ident_d)

            T = constp.tile([P, F], f16)
            R = constp.tile([P, F], f16)
            nc.gpsimd.memset(T[:], 1.0)
            part = slabp.tile([P, NG, 4, F], f16)
            nc.gpsimd.memset(part[:, :, 0], 0.0)
            ps1 = slabp.tile([P, 4, F], f16)
            qs1 = slabp.tile([P, 2, F], f16)

            # ACT warmup: trigger activation-table load while input DMA runs
            warm = constp.tile([P, 8], f16)
            nc.gpsimd.memset(warm[:], 0.5)
            wout = constp.tile([P, 8], f16)
            nc.scalar.sign(wout[:], warm[:])
            nc.scalar.activation(wout[:], warm[:], act.Relu, bias=1.0, scale=-1.0)

            # ---- input DMAs: alpha in (4, 4, 12)-stroke chunks first, then
            # per-quad colors, all on the Sync queue.
            ae1 = slabp.tile([P, 8, F], f16)
            ae2 = slabp.tile([P, S2, F], f16)
            nc.sync.dma_start(
                ae1[:, 0:4], ae_d[:, : 4 * F].rearrange("p (s f) -> p s f", s=4)
            )
            nc.sync.dma_start(
                ae1[:, 4:8],
                ae_d[:, 4 * F : 8 * F].rearrange("p (s f) -> p s f", s=4),
            )
            nc.sync.dma_start(
                ae2[:], ae_d[:, 8 * F :].rearrange("p (s f) -> p s f", s=S2)
            )
            ctile = slabp.tile([P, depth, 3, F], f16)
            for qi in range(NQ):
                lo = qi * 4 * 3 * F
                nc.sync.dma_start(
                    ctile[:, 4 * qi : 4 * qi + 4],
                    c_d[:, lo : lo + 4 * 3 * F].rearrange(
                        "p (s c f) -> p s c f", s=4, c=3
                    ),
                )

            # ---- serial chain body (emitted interleaved with the precompute
            # so quad 0 runs while later inputs are still in flight) ----
            cacc = psump.tile([P, 3 * F], f32)
            tail = constp.tile([P, 3, F], f16)

            def serial_quad(qi):
                gated = qi >= 2
                bQd = (b1 if qi < 2 else b2t)
                lq = qi if qi < 2 else qi - 2
                b_quad = bQd[:, 4 * lq : 4 * lq + 4]
                T_b = T[:].unsqueeze(1).broadcast_to((P, 4, F))
                T_b2 = T[:].unsqueeze(1).broadcast_to((P, 2, F))
                ta = workp.tile([P, 4, F], f16, tag="ta")
                if not gated:
                    nc.vector.tensor_tensor(ta[:], b_quad, T_b, op=op.mult)
                elif qi == 2:
                    # strokes 8,9 can't be gated (R <= 9): plain; gate 10,11
                    nc.vector.tensor_tensor(
                        ta[:, 0:2], b_quad[:, 0:2], T_b2, op=op.mult
                    )
                    tmp = workp.tile([P, 2, F], f16, tag="tmp")
                    R_b2 = R[:].unsqueeze(1).broadcast_to((P, 2, F))
                    nc.vector.tensor_tensor(
                        tmp[:], part[:, 0, 2:4], R_b2, op=op.add
                    )
                    mb = workp.tile([P, 2, F], f16, tag="mb")
                    nc.vector.scalar_tensor_tensor(
                        mb[:], tmp[:], float(K) - 0.5, b_quad[:, 2:4],
                        op0=op.is_lt, op1=op.mult,
                    )
                    nc.vector.tensor_tensor(ta[:, 2:4], mb[:], T_b2, op=op.mult)
                else:
                    tmp = workp.tile([P, 4, F], f16, tag="tmp4")
                    R_b = R[:].unsqueeze(1).broadcast_to((P, 4, F))
                    nc.vector.tensor_tensor(tmp[:], part[:, lq], R_b, op=op.add)
                    mb = workp.tile([P, 4, F], f16, tag="mb4")
                    nc.vector.scalar_tensor_tensor(
                        mb[:], tmp[:], float(K) - 0.5, b_quad,
                        op0=op.is_lt, op1=op.mult,
                    )
                    nc.vector.tensor_tensor(ta[:], mb[:], T_b, op=op.mult)

                if qi == 1:
                    # fill the wait for quad-1's color DMA with the chunk-1
                    # count ops (pass1 is ready: it rides first on ACT)
                    p1P = pass1[:].rearrange("p (pr two) f -> p pr two f", two=2)
                    nc.vector.tensor_tensor(
                        ps1[:], p1P[:, :, 0], p1P[:, :, 1], op=op.add
                    )
                    ps1P = ps1[:].rearrange("p (qd two) f -> p qd two f", two=2)
                    nc.vector.tensor_tensor(
                        qs1[:], ps1P[:, :, 0], ps1P[:, :, 1], op=op.add
                    )
                    nc.vector.tensor_tensor(
                        R[:], qs1[:, 0], qs1[:, 1], op=op.add
                    )

                prod = prodp.tile([P, 4, 3, F], f16, tag="prod")
                ta_b = ta[:].unsqueeze(2).broadcast_to((P, 4, 3, F))
                nc.vector.tensor_tensor(
                    prod[:], ctile[:, 4 * qi : 4 * qi + 4], ta_b, op=op.mult
                )
                # last quad: only 2 strokes via PE (PSUM group closes early so
                # PE drains in the shadow); strokes depth-2, depth-1 summed on
                # DVE into `tail`.
                nmm = 2 if qi == NQ - 1 else 4
                for j in range(nmm):
                    s = 4 * qi + j
                    nc.tensor.matmul(
                        cacc[:], ident[:],
                        prod[:, j].rearrange("p c f -> p (c f)"),
                        start=(s == 0), stop=(s == 4 * (NQ - 1) + 1),
                    )
                if qi == NQ - 1:
                    nc.vector.tensor_tensor(
                        tail[:], prod[:, 2], prod[:, 3], op=op.add
                    )

                # T update (after prods consumed ta)
                if not gated:
                    nc.vector.tensor_tensor(T[:], T[:], qq1[:, qi], op=op.mult)
                else:
                    h = workp.tile([P, 2, F], f16, tag="h")
                    nc.vector.tensor_tensor(
                        h[:], ta[:, 0:2], ta[:, 2:4], op=op.add
                    )
                    nc.vector.tensor_tensor(T[:], T[:], h[:, 0], op=op.subtract)
                    nc.vector.tensor_tensor(T[:], T[:], h[:, 1], op=op.subtract)

                # R update (raw pass count at next quad start); the qi==1
                # init lives in the gate-machinery section (qs1 exists there)
                if gated and qi < NQ - 1:
                    nc.vector.tensor_tensor(
                        R[:], R[:], qs2[:, lq], op=op.add
                    )

            # ---- q = 1 - ae: quads 0,1 on DVE (the serial chain must not
            # wait on the ACT queue); strokes 8+ on ACT.
            q1 = slabp.tile([P, 8, F], f16)
            nc.vector.tensor_scalar(
                q1[:, 0:4], ae1[:, 0:4], -1.0, 1.0, op0=op.mult, op1=op.add
            )
            # E tiles: per-quad exclusive prefix products of q
            # (pos0 = 1 via early memset, pos1 = q0, pos2 = q0q1, pos3 = q0q1q2)
            # so all b-terms come from ONE 2x multiply b = ae * E per chunk.
            E1 = slabp.tile([P, 8, F], f16)
            E2 = slabp.tile([P, S2, F], f16)
            EQ1 = E1[:].rearrange("p (qd s) f -> p qd s f", s=4)
            EQ2 = E2[:].rearrange("p (qd s) f -> p qd s f", s=4)
            nc.gpsimd.memset(EQ1[:, :, 0], 1.0)
            nc.gpsimd.memset(EQ2[:, :, 0], 1.0)
            qQ1 = q1[:].rearrange("p (qd s) f -> p qd s f", s=4)

            # ACT queue order: pass1 first (it's ready as soon as the first
            # two alpha chunks land, and the DVE fills its c1-DMA wait with
            # the pass1-derived count ops), then q2 (alpha-DMA-bound anyway),
            # then the late gate inputs.  (E pos-1 copies for chunk 1 are on
            # DVE: an ACT copy emitted before the DVE q1 write would
            # read-before-write.)
            pass1 = slabp.tile([P, 8, F], f16)
            pass2 = slabp.tile([P, S2 - 1, F], f16)
            nc.scalar.sign(pass1[:], ae1[:])
            q2 = slabp.tile([P, S2, F], f16)
            nc.scalar.activation(q2[:], ae2[:], act.Relu, bias=1.0, scale=-1.0)
            qQ2 = q2[:].rearrange("p (qd s) f -> p qd s f", s=4)
            nc.scalar.copy(EQ2[:, :, 1], qQ2[:, :, 0])
            nc.scalar.sign(pass2[:], ae2[:, : S2 - 1])

            # ---- DVE side of the E / b precompute.  Quad 0's closure only
            # needs the first 4-stroke alpha chunk, so its serial work fills
            # the wait for the second chunk's DMA.
            b1 = slabp.tile([P, 8, F], f16)
            b2t = slabp.tile([P, S2, F], f16)
            qq1 = slabp.tile([P, 2, F], f16)
            q12od = workp.tile([P, 2, F], f16, tag="q12od")
            for lq in range(2):
                sl = slice(lq, lq + 1)
                if lq == 1:  # q for quad 1 (waits on the 2nd alpha chunk)
                    nc.vector.tensor_scalar(
                        q1[:, 4:8], ae1[:, 4:8], -1.0, 1.0,
                        op0=op.mult, op1=op.add,
                    )
                nc.vector.tensor_scalar(
                    EQ1[:, sl, 1], qQ1[:, sl, 0], 1.0, None, op0=op.mult
                )
                nc.vector.tensor_tensor(
                    EQ1[:, sl, 2], qQ1[:, sl, 0], qQ1[:, sl, 1], op=op.mult
                )
                nc.vector.tensor_tensor(
                    q12od[:, sl], qQ1[:, sl, 2], qQ1[:, sl, 3], op=op.mult
                )
                nc.vector.tensor_tensor(
                    qq1[:, sl], EQ1[:, sl, 2], q12od[:, sl], op=op.mult
                )
                nc.vector.tensor_tensor(
                    EQ1[:, sl, 3], EQ1[:, sl, 2], qQ1[:, sl, 2], op=op.mult
                )
                nc.vector.tensor_tensor(
                    b1[:, 4 * lq : 4 * lq + 4],
                    ae1[:, 4 * lq : 4 * lq + 4],
                    E1[:, 4 * lq : 4 * lq + 4], op=op.mult,
                )
                if lq == 0:
                    serial_quad(0)
            serial_quad(1)
            # chunk 2 (quads 2..)
            nc.vector.tensor_tensor(
                EQ2[:, :, 2], qQ2[:, :, 0], qQ2[:, :, 1], op=op.mult
            )
            nc.vector.tensor_tensor(
                EQ2[:, :, 3], EQ2[:, :, 2], qQ2[:, :, 2], op=op.mult
            )
            nc.vector.tensor_tensor(b2t[:], ae2[:], E2[:], op=op.mult)

            # ---- gate machinery (batched): pair/quad sums of pass bits,
            # intra-quad partial prefixes for gated quads.
            npair2 = (S2 - 2) // 2
            ps2 = slabp.tile([P, npair2, F], f16)
            p2P = pass2[:, : 2 * npair2].rearrange(
                "p (pr two) f -> p pr two f", two=2
            )
            nc.vector.tensor_tensor(ps2[:], p2P[:, :, 0], p2P[:, :, 1], op=op.add)

            qs2 = slabp.tile([P, NG - 1, F], f16)
            ps2P = ps2[:, : 2 * (NG - 1)].rearrange(
                "p (qd two) f -> p qd two f", two=2
            )
            nc.vector.tensor_tensor(qs2[:], ps2P[:, :, 0], ps2P[:, :, 1], op=op.add)

            # partials: j=0: 0; j=1: p0; j=2: p0+p1; j=3: p0+p1+p2 per quad
            nc.scalar.copy(part[:, :, 1], pass2[:, 0::4])     # strokes 8,12,16
            nc.scalar.copy(part[:, :, 2], ps2[:, 0::2])       # pairs 4,6,8
            nc.vector.tensor_tensor(
                part[:, :, 3], ps2[:, 0::2], pass2[:, 2::4], op=op.add
            )

            # ---- gated serial quads ----
            for qi in range(2, NQ):
                serial_quad(qi)

            # out = cacc + (tail + T): the tail+T fold runs while PE still
            # drains; only the last adds depend on the final PSUM state.  Two
            # halves so the first DMA's issue overlaps the second add.
            nc.vector.tensor_tensor(
                tail[:], tail[:],
                T[:].unsqueeze(1).broadcast_to((P, 3, F)), op=op.add,
            )
            outt = constp.tile([P, 3, F], f16)
            caccv = cacc[:].rearrange("p (c f) -> p c f", c=3)
            nc.vector.tensor_tensor(
                outt[:, 0:2], caccv[:, 0:2], tail[:, 0:2], op=op.add
            )
            nc.sync.dma_start(
                out_d[:, : 2 * F], outt[:, 0:2].rearrange("p c f -> p (c f)")
            )
            nc.vector.tensor_tensor(
                outt[:, 2:3], caccv[:, 2:3], tail[:, 2:3], op=op.add
            )
            nc.sync.dma_start(
                out_d[:, 2 * F :], outt[:, 2:3].rearrange("p c f -> p (c f)")
            )

    nc.compile()
    return nc


def _prep_inputs(color_stroke, alpha, depth):
    """Slice the top `depth` strokes (reversed so stroke 0 = highest index),
    mask alpha by the fp32-exact threshold on host, convert to fp16 and lay
    out per core: ae [P, depth*F], color [P, depth*3*F]."""
    a_r = alpha[:, N - depth :, 0][:, ::-1]          # (B, depth, P, F)
    ae_host = (a_r * (a_r > ALPHA_THRESH)).astype(np.float16)
    c_r = color_stroke[:, N - depth :][:, ::-1].astype(np.float16)  # (B,depth,3,P,F)
    ident = np.eye(P, dtype=np.float16)
    in_maps = []
    for b in range(B):
        ae_core = np.ascontiguousarray(ae_host[b].transpose(1, 0, 2)).reshape(
            P, depth * F
        )
        c_core = np.ascontiguousarray(c_r[b].transpose(2, 0, 1, 3)).reshape(
            P, depth * 3 * F
        )
        in_maps.append({"ae_in": ae_core, "color_in": c_core, "ident_in": ident})
    return in_maps


def _reference_numpy(color_stroke, alpha):
    """Exact replication of the oracle (incl. top-k tie-breaking) on host.
    Only used when the depth-cutoff precondition fails (pathological inputs)."""
    stroke_ids = np.arange(1, N + 1, dtype=np.int32).reshape(1, N, 1, 1)
    draw = stroke_ids * (alpha[:, :, 0] > ALPHA_THRESH).astype(np.int32)  # (B,N,W,W)
    draw_t = np.moveaxis(draw, 1, -1)  # (B,W,W,N)
    idx = np.argsort(-draw_t, axis=-1, kind="stable")[..., :K]  # (B,W,W,K)
    idx = np.moveaxis(idx, -1, 1)[:, :, None]  # (B,K,1,W,W)
    alpha_k = np.take_along_axis(alpha, idx, axis=1)  # (B,K,1,W,W)
    color_k = np.take_along_axis(color_stroke, idx, axis=1)  # (B,K,3,W,W)
    canvas = np.ones((B, 3, W, W), dtype=color_stroke.dtype)
    for i in range(K - 1, -1, -1):
        a = alpha_k[:, i]
        canvas = canvas * (1.0 - a) + a * color_k[:, i]
    return canvas


def kernel(color_stroke, alpha):
    color_stroke = np.asarray(color_stroke, dtype=np.float32)
    alpha = np.asarray(alpha, dtype=np.float32)
    assert color_stroke.shape == (B, N, 3, W, W), color_stroke.shape
    assert alpha.shape == (B, N, 1, W, W), alpha.shape

    # Precondition for the depth cutoff: every pixel finds its 10 passing
    # strokes within the top D.
    top_pass = (alpha[:, N - D :, 0] > ALPHA_THRESH).sum(axis=1)
    if top_pass.min() < K:
        return _reference_numpy(color_stroke, alpha)

    from concourse.bass_utils import run_bass_kernel_spmd

    if D not in _nc_cache:
        _nc_cache[D] = _build_nc(D)
    nc = _nc_cache[D]

    in_maps = _prep_inputs(color_stroke, alpha, D)
    res = run_bass_kernel_spmd(nc, in_maps, core_ids=list(range(NCORES)))

    out = np.empty((B, 3, W, W), dtype=np.float32)
    for b in range(B):
        out[b] = (
            res.results[b]["out"].reshape(P, 3, F).transpose(1, 0, 2)
            .astype(np.float32)
        )
    return out


# revision 34
# speedup vs baseline: 1.0146x; 1.0146x over previous
"""Trainium2 Bass kernel for AttnPainterOil-style top-K stroke compositing.

Problem semantics (per pixel, fully independent):
  draw[n] = (n+1) * (alpha[n] > 0.1); top-K=10 of draw over N=256 strokes
  (descending) == the 10 highest-index strokes with alpha > 0.1.  Gather
  alpha/color at those indices and composite back-to-front over white.

Device formulation (front-to-back, strokes in descending index order):
maintain per-pixel transmittance T (init 1) and a raw pass-count R.  For
stroke s with host-masked alpha ae_s (= a * 1{a > 0.1}, fp32-exact mask
applied on host, shipped as fp16):
  gate m_s = 1{R_s < 10} with R_s = #passing among strokes < s (RAW count,
  independent of gating -- so it batches), ta_s = m_s * b_s * T_quad, where
  b_s are quad-local exclusive-prefix weights b_j = ae_j * prod_{i<j}(1-ae_i)
  (batch-precomputed).  Within a quad the gate mask is a suffix cut, so
  ta_j = m_j * b_j * T is exact and T_new = T - sum_j ta_j.

Scheduling facts measured on TRN2 for this kernel family:
  - DVE op = ~150ns dispatch + free_elems cycles; fp16 unit-stride runs 2x
    (STT runs 1x).  Broadcasts on outer dims keep 2x.
  - The ACT (scalar) queue is a serial second critical path: anything the
    serial chain needs (q for the first quads, b pos-0) must NOT ride it.
    ACT keeps only late-needed work: q for strokes 8+, pass bits, partial
    copies.
  - Each dma_start costs ~700ns issue on its queue engine; alpha goes first
    in small chunks, colors follow per-quad on the same Sync queue
    (concurrent big transfers on another queue starve the alpha DMA).
  - fp16 everywhere passes easily (measured 1.0e-3 vs 2e-2 tolerance).
  - Depth 20 suffices for this input (every pixel's 10th passing stroke is
    within the top 20; checked on host, exact numpy fallback otherwise).
  - PE accumulates ta*c into PSUM via fp16 identity matmuls.

Sharding: pure data parallel, one batch element per NeuronCore (B=8).
"""

import numpy as np

B, N, W, K = 8, 256, 128, 10
ALPHA_THRESH = 0.1
D = 20          # strokes processed from the top (host-verified sufficient)
P = 128         # partitions (pixel rows)
F = 128         # free dim (pixel cols)
NCORES = 8

_nc_cache = {}


def _build_nc(depth):
    import concourse.bass as bass  # noqa: F401
    import concourse.tile as tile
    from concourse import bacc, mybir
    from concourse.vector_clock import ScopedClock

    op = mybir.AluOpType
    act = mybir.ActivationFunctionType
    f16 = mybir.dt.float16
    f32 = mybir.dt.float32

    assert depth % 4 == 0
    NQ = depth // 4          # quads (5)
    NG = NQ - 2              # gated quads (strokes 8..depth-1)
    S2 = depth - 8           # strokes in the ACT-side alpha chunk

    class _OneShotTileContext(tile.TileContext):
        """TileContext with a slim exit: the drain alone (it waits on the
        global clock, including output-DMA completion) -- no all-engine
        barriers and no per-semaphore clears."""

        def _drain_and_barrier(self, tick_clock, wait_clock):
            drain_inst = self.nc.sync.drain()
            wait_clock.add_sem_waits(
                drain_inst.ins, ScopedClock({None: tick_clock.global_clock})
            )
            popped = self.nc._tile_sem_poison_stack.pop()
            assert popped is self._sem_poison

    nc = bacc.Bacc("TRN2", target_bir_lowering=False, debug=False)

    ae_d = nc.dram_tensor("ae_in", [P, depth * F], f16, kind="ExternalInput").ap()
    c_d = nc.dram_tensor("color_in", [P, depth * 3 * F], f16, kind="ExternalInput").ap()
    ident_d = nc.dram_tensor("ident_in", [P, P], f16, kind="ExternalInput").ap()
    out_d = nc.dram_tensor("out", [P, 3 * F], f16, kind="ExternalOutput").ap()

    with _OneShotTileContext(nc) as tc:
        with (
            tc.tile_pool(name="const", bufs=1) as constp,
            tc.tile_pool(name="slab", bufs=1) as slabp,
            tc.tile_pool(name="work", bufs=2) as workp,
            tc.tile_pool(name="prodp", bufs=2) as prodp,
            tc.tile_pool(name="psum", bufs=1, space="PSUM") as psump,
        ):
            # ident via SWDGE (gpsimd queue) so it doesn't delay the HWDGE
            # input stream; it's only needed by the first matmul.
            ident = constp.tile([P, P], f16)
            nc.gpsimd.dma_start(ident[:], ident_d)

            T = constp.tile([P, F], f16)
            R = constp.tile([P, F], f16)
            nc.gpsimd.memset(T[:], 1.0)
            part = slabp.tile([P, NG, 4, F], f16)
            nc.gpsimd.memset(part[:, :, 0], 0.0)
            ps1 = slabp.tile([P, 4, F], f16)
            qs1 = slabp.tile([P, 2, F], f16)

            # ACT warmup: trigger activation-table load while input DMA runs
            warm = constp.tile([P, 8], f16)
            nc.gpsimd.memset(warm[:], 0.5)
            wout = constp.tile([P, 8], f16)
            nc.scalar.sign(wout[:], warm[:])
            nc.scalar.activation(wout[:], warm[:], act.Relu, bias=1.0, scale=-1.0)

            # ---- input DMAs: alpha in (4, 4, 12)-stroke chunks first, then
            # per-quad colors, all on the Sync queue.
            ae1 = slabp.tile([P, 8, F], f16)
            ae2 = slabp.tile([P, S2, F], f16)
            nc.sync.dma_start(
                ae1[:, 0:4], ae_d[:, : 4 * F].rearrange("p (s f) -> p s f", s=4)
            )
            nc.sync.dma_start(
                ae1[:, 4:8],
                ae_d[:, 4 * F : 8 * F].rearrange("p (s f) -> p s f", s=4),
            )
            nc.sync.dma_start(
                ae2[:], ae_d[:, 8 * F :].rearrange("p (s f) -> p s f", s=S2)
            )
            ctile = slabp.tile([P, depth, 3, F], f16)
            for qi in range(NQ):
                lo = qi * 4 * 3 * F
                nc.sync.dma_start(
                    ctile[:, 4 * qi : 4 * qi + 4],
                    c_d[:, lo : lo + 4 * 3 * F].rearrange(
                        "p (s c f) -> p s c f", s=4, c=3
                    ),
                )

            # ---- serial chain body (emitted interleaved with the precompute
            # so quad 0 runs while later inputs are still in flight) ----
            cacc = psump.tile([P, 3 * F], f32)
            tail = constp.tile([P, 3, F], f16)

            def serial_quad(qi):
                gated = qi >= 2
                bQd = (b1 if qi < 2 else b2t)
                lq = qi if qi < 2 else qi - 2
                b_quad = bQd[:, 4 * lq : 4 * lq + 4]
                T_b = T[:].unsqueeze(1).broadcast_to((P, 4, F))
                T_b2 = T[:].unsqueeze(1).broadcast_to((P, 2, F))
                ta = workp.tile([P, 4, F], f16, tag="ta")
                if not gated:
                    nc.vector.tensor_tensor(ta[:], b_quad, T_b, op=op.mult)
                elif qi == 2:
                    # strokes 8,9 can't be gated (R <= 9): plain; gate 10,11
                    nc.vector.tensor_tensor(
                        ta[:, 0:2], b_quad[:, 0:2], T_b2, op=op.mult
                    )
                    tmp = workp.tile([P, 2, F], f16, tag="tmp")
                    R_b2 = R[:].unsqueeze(1).broadcast_to((P, 2, F))
                    nc.vector.tensor_tensor(
                        tmp[:], part[:, 0, 2:4], R_b2, op=op.add
                    )
                    mb = workp.tile([P, 2, F], f16, tag="mb")
                    nc.vector.scalar_tensor_tensor(
                        mb[:], tmp[:], float(K) - 0.5, b_quad[:, 2:4],
                        op0=op.is_lt, op1=op.mult,
                    )
                    nc.vector.tensor_tensor(ta[:, 2:4], mb[:], T_b2, op=op.mult)
                else:
                    tmp = workp.tile([P, 4, F], f16, tag="tmp4")
                    R_b = R[:].unsqueeze(1).broadcast_to((P, 4, F))
                    nc.vector.tensor_tensor(tmp[:], part[:, lq], R_b, op=op.add)
                    mb = workp.tile([P, 4, F], f16, tag="mb4")
                    nc.vector.scalar_tensor_tensor(
                        mb[:], tmp[:], float(K) - 0.5, b_quad,
                        op0=op.is_lt, op1=op.mult,
                    )
                    nc.vector.tensor_tensor(ta[:], mb[:], T_b, op=op.mult)

                if qi == 1:
                    # fill the wait for quad-1's color DMA with the chunk-1
                    # count ops (pass1 is ready: it rides first on ACT)
                    p1P = pass1[:].rearrange("p (pr two) f -> p pr two f", two=2)
                    nc.vector.tensor_tensor(
                        ps1[:], p1P[:, :, 0], p1P[:, :, 1], op=op.add
                    )
                    ps1P = ps1[:].rearrange("p (qd two) f -> p qd two f", two=2)
                    nc.vector.tensor_tensor(
                        qs1[:], ps1P[:, :, 0], ps1P[:, :, 1], op=op.add
                    )
                    nc.vector.tensor_tensor(
                        R[:], qs1[:, 0], qs1[:, 1], op=op.add
                    )

                prod = prodp.tile([P, 4, 3, F], f16, tag="prod")
                ta_b = ta[:].unsqueeze(2).broadcast_to((P, 4, 3, F))
                nc.vector.tensor_tensor(
                    prod[:], ctile[:, 4 * qi : 4 * qi + 4], ta_b, op=op.mult
                )
                # last quad: only 2 strokes via PE (PSUM group closes early so
                # PE drains in the shadow); strokes depth-2, depth-1 summed on
                # DVE into `tail`.
                nmm = 2 if qi == NQ - 1 else 4
                for j in range(nmm):
                    s = 4 * qi + j
                    nc.tensor.matmul(
                        cacc[:], ident[:],
                        prod[:, j].rearrange("p c f -> p (c f)"),
                        start=(s == 0), stop=(s == 4 * (NQ - 1) + 1),
                    )
                if qi == NQ - 1:
                    nc.vector.tensor_tensor(
                        tail[:], prod[:, 2], prod[:, 3], op=op.add
                    )

                # T update (after prods consumed ta)
                if not gated:
                    nc.vector.tensor_tensor(T[:], T[:], qq1[:, qi], op=op.mult)
                else:
                    h = workp.tile([P, 2, F], f16, tag="h")
                    nc.vector.tensor_tensor(
                        h[:], ta[:, 0:2], ta[:, 2:4], op=op.add
                    )
                    nc.vector.tensor_tensor(T[:], T[:], h[:, 0], op=op.subtract)
                    nc.vector.tensor_tensor(T[:], T[:], h[:, 1], op=op.subtract)

                # R update (raw pass count at next quad start); the qi==1
                # init lives in the gate-machinery section (qs1 exists there)
                if gated and qi < NQ - 1:
                    nc.vector.tensor_tensor(
                        R[:], R[:], qs2[:, lq], op=op.add
                    )

            # ---- q = 1 - ae: quads 0,1 on DVE (the serial chain must not
            # wait on the ACT queue); strokes 8+ on ACT.
            q1 = slabp.tile([P, 8, F], f16)
            nc.vector.tensor_scalar(
                q1[:, 0:4], ae1[:, 0:4], -1.0, 1.0, op0=op.mult, op1=op.add
            )
            # E tiles: per-quad exclusive prefix products of q
            # (pos0 = 1 via early memset, pos1 = q0, pos2 = q0q1, pos3 = q0q1q2)
            # so all b-terms come from ONE 2x multiply b = ae * E per chunk.
            E1 = slabp.tile([P, 8, F], f16)
            E2 = slabp.tile([P, S2, F], f16)
            EQ1 = E1[:].rearrange("p (qd s) f -> p qd s f", s=4)
            EQ2 = E2[:].rearrange("p (qd s) f -> p qd s f", s=4)
            nc.gpsimd.memset(EQ1[:, :, 0], 1.0)
            nc.gpsimd.memset(EQ2[:, :, 0], 1.0)
            qQ1 = q1[:].rearrange("p (qd s) f -> p qd s f", s=4)

            # ACT queue order: pass1 first (it's ready as soon as the first
            # two alpha chunks land, and the DVE fills its c1-DMA wait with
            # the pass1-derived count ops), then q2 (alpha-DMA-bound anyway),
            # then the late gate inputs.  (E pos-1 copies for chunk 1 are on
            # DVE: an ACT copy emitted before the DVE q1 write would
            # read-before-write.)
            pass1 = slabp.tile([P, 8, F], f16)
            pass2 = slabp.tile([P, S2 - 1, F], f16)
            nc.scalar.sign(pass1[:], ae1[:])
            q2 = slabp.tile([P, S2, F], f16)
            nc.scalar.activation(q2[:], ae2[:], act.Relu, bias=1.0, scale=-1.0)
            qQ2 = q2[:].rearrange("p (qd s) f -> p qd s f", s=4)
            nc.scalar.copy(EQ2[:, :, 1], qQ2[:, :, 0])
            nc.scalar.sign(pass2[:], ae2[:, : S2 - 1])

            # ---- DVE side of the E / b precompute.  Quad 0's closure only
            # needs the first 4-stroke alpha chunk, so its serial work fills
            # the wait for the second chunk's DMA.
            b1 = slabp.tile([P, 8, F], f16)
            b2t = slabp.tile([P, S2, F], f16)
            qq1 = slabp.tile([P, 2, F], f16)
            q12od = workp.tile([P, 2, F], f16, tag="q12od")
            for lq in range(2):
                sl = slice(lq, lq + 1)
                if lq == 1:  # q for quad 1 (waits on the 2nd alpha chunk)
                    nc.vector.tensor_scalar(
                        q1[:, 4:8], ae1[:, 4:8], -1.0, 1.0,
                        op0=op.mult, op1=op.add,
                    )
                nc.vector.tensor_scalar(
                    EQ1[:, sl, 1], qQ1[:, sl, 0], 1.0, None, op0=op.mult
                )
                nc.vector.tensor_tensor(
                    EQ1[:, sl, 2], qQ1[:, sl, 0], qQ1[:, sl, 1], op=op.mult
                )
                nc.vector.tensor_tensor(
                    q12od[:, sl], qQ1[:, sl, 2], qQ1[:, sl, 3], op=op.mult
                )
                nc.vector.tensor_tensor(
                    qq1[:, sl], EQ1[:, sl, 2], q12od[:, sl], op=op.mult
                )
                nc.vector.tensor_tensor(
                    EQ1[:, sl, 3], EQ1[:, sl, 2], qQ1[:, sl, 2], op=op.mult
                )
                nc.vector.tensor_tensor(
                    b1[:, 4 * lq : 4 * lq + 4],
                    ae1[:, 4 * lq : 4 * lq + 4],
                    E1[:, 4 * lq : 4 * lq + 4], op=op.mult,
                )
                if lq == 0:
                    serial_quad(0)
            serial_quad(1)
            # chunk 2 (quads 2..)
            nc.vector.tensor_tensor(
                EQ2[:, :, 2], qQ2[:, :, 0], qQ2[:, :, 1], op=op.mult
            )
            nc.vector.tensor_tensor(
                EQ2[:, :, 3], EQ2[:, :, 2], qQ2[:, :, 2], op=op.mult
            )
            nc.vector.tensor_tensor(b2t[:], ae2[:], E2[:], op=op.mult)

            # ---- gate machinery (batched): pair/quad sums of pass bits,
            # intra-quad partial prefixes for gated quads.
            npair2 = (S2 - 2) // 2
            ps2 = slabp.tile([P, npair2, F], f16)
            p2P = pass2[:, : 2 * npair2].rearrange(
                "p (pr two) f -> p pr two f", two=2
            )
            nc.vector.tensor_tensor(ps2[:], p2P[:, :, 0], p2P[:, :, 1], op=op.add)

            qs2 = slabp.tile([P, NG - 1, F], f16)
            ps2P = ps2[:, : 2 * (NG - 1)].rearrange(
                "p (qd two) f -> p qd two f", two=2
            )
            nc.vector.tensor_tensor(qs2[:], ps2P[:, :, 0], ps2P[:, :, 1], op=op.add)

            # partials: j=0: 0; j=1: p0; j=2: p0+p1; j=3: p0+p1+p2 per quad
            nc.scalar.copy(part[:, :, 1], pass2[:, 0::4])     # strokes 8,12,16
            nc.scalar.copy(part[:, :, 2], ps2[:, 0::2])       # pairs 4,6,8
            nc.vector.tensor_tensor(
                part[:, :, 3], ps2[:, 0::2], pass2[:, 2::4], op=op.add
            )

            # ---- gated serial quads ----
            for qi in range(2, NQ):
                serial_quad(qi)

            # out = cacc + (tail + T): the tail+T fold runs while PE still
            # drains; only the last adds depend on the final PSUM state.  Two
            # halves so the first DMA's issue overlaps the second add.
            nc.vector.tensor_tensor(
                tail[:], tail[:],
                T[:].unsqueeze(1).broadcast_to((P, 3, F)), op=op.add,
            )
            outt = constp.tile([P, 3, F], f16)
            caccv = cacc[:].rearrange("p (c f) -> p c f", c=3)
            nc.vector.tensor_tensor(
                outt[:, 0:2], caccv[:, 0:2], tail[:, 0:2], op=op.add
            )
            nc.sync.dma_start(
                out_d[:, : 2 * F], outt[:, 0:2].rearrange("p c f -> p (c f)")
            )
            nc.vector.tensor_tensor(
                outt[:, 2:3], caccv[:, 2:3], tail[:, 2:3], op=op.add
            )
            # second half rides the idle GpSimd queue so its ~0.6us issue
            # overlaps the first half's issue on Sync
            nc.gpsimd.dma_start(
                out_d[:, 2 * F :], outt[:, 2:3].rearrange("p c f -> p (c f)")
            )

    nc.compile()
    return nc


def _prep_inputs(color_stroke, alpha, depth):
    """Slice the top `depth` strokes (reversed so stroke 0 = highest index),
    mask alpha by the fp32-exact threshold on host, convert to fp16 and lay
    out per core: ae [P, depth*F], color [P, depth*3*F]."""
    a_r = alpha[:, N - depth :, 0][:, ::-1]          # (B, depth, P, F)
    ae_host = (a_r * (a_r > ALPHA_THRESH)).astype(np.float16)
    c_r = color_stroke[:, N - depth :][:, ::-1].astype(np.float16)  # (B,depth,3,P,F)
    ident = np.eye(P, dtype=np.float16)
    in_maps = []
    for b in range(B):
        ae_core = np.ascontiguousarray(ae_host[b].transpose(1, 0, 2)).reshape(
            P, depth * F
        )
        c_core = np.ascontiguousarray(c_r[b].transpose(2, 0, 1, 3)).reshape(
            P, depth * 3 * F
        )
        in_maps.append({"ae_in": ae_core, "color_in": c_core, "ident_in": ident})
    return in_maps


def _reference_numpy(color_stroke, alpha):
    """Exact replication of the oracle (incl. top-k tie-breaking) on host.
    Only used when the depth-cutoff precondition fails (pathological inputs)."""
    stroke_ids = np.arange(1, N + 1, dtype=np.int32).reshape(1, N, 1, 1)
    draw = stroke_ids * (alpha[:, :, 0] > ALPHA_THRESH).astype(np.int32)  # (B,N,W,W)
    draw_t = np.moveaxis(draw, 1, -1)  # (B,W,W,N)
    idx = np.argsort(-draw_t, axis=-1, kind="stable")[..., :K]  # (B,W,W,K)
    idx = np.moveaxis(idx, -1, 1)[:, :, None]  # (B,K,1,W,W)
    alpha_k = np.take_along_axis(alpha, idx, axis=1)  # (B,K,1,W,W)
    color_k = np.take_along_axis(color_stroke, idx, axis=1)  # (B,K,3,W,W)
    canvas = np.ones((B, 3, W, W), dtype=color_stroke.dtype)
    for i in range(K - 1, -1, -1):
        a = alpha_k[:, i]
        canvas = canvas * (1.0 - a) + a * color_k[:, i]
    return canvas


def kernel(color_stroke, alpha):
    color_stroke = np.asarray(color_stroke, dtype=np.float32)
    alpha = np.asarray(alpha, dtype=np.float32)
    assert color_stroke.shape == (B, N, 3, W, W), color_stroke.shape
    assert alpha.shape == (B, N, 1, W, W), alpha.shape

    # Precondition for the depth cutoff: every pixel finds its 10 passing
    # strokes within the top D.
    top_pass = (alpha[:, N - D :, 0] > ALPHA_THRESH).sum(axis=1)
    if top_pass.min() < K:
        return _reference_numpy(color_stroke, alpha)

    from concourse.bass_utils import run_bass_kernel_spmd

    if D not in _nc_cache:
        _nc_cache[D] = _build_nc(D)
    nc = _nc_cache[D]

    in_maps = _prep_inputs(color_stroke, alpha, D)
    res = run_bass_kernel_spmd(nc, in_maps, core_ids=list(range(NCORES)))

    out = np.empty((B, 3, W, W), dtype=np.float32)
    for b in range(B):
        out[b] = (
            res.results[b]["out"].reshape(P, 3, F).transpose(1, 0, 2)
            .astype(np.float32)
        )
    return out


# revision 38
# speedup vs baseline: 1.0355x; 1.0206x over previous
"""Trainium2 Bass kernel for AttnPainterOil-style top-K stroke compositing.

Problem semantics (per pixel, fully independent):
  draw[n] = (n+1) * (alpha[n] > 0.1); top-K=10 of draw over N=256 strokes
  (descending) == the 10 highest-index strokes with alpha > 0.1.  Gather
  alpha/color at those indices and composite back-to-front over white.

Device formulation (front-to-back, strokes in descending index order):
maintain per-pixel transmittance T (init 1) and a raw pass-count R.  For
stroke s with host-masked alpha ae_s (= a * 1{a > 0.1}, fp32-exact mask
applied on host, shipped as fp16):
  gate m_s = 1{R_s < 10} with R_s = #passing among strokes < s (RAW count,
  independent of gating -- so it batches), ta_s = m_s * b_s * T_quad, where
  b_s are quad-local exclusive-prefix weights b_j = ae_j * prod_{i<j}(1-ae_i)
  (batch-precomputed).  Within a quad the gate mask is a suffix cut, so
  ta_j = m_j * b_j * T is exact and T_new = T - sum_j ta_j.

Scheduling facts measured on TRN2 for this kernel family:
  - DVE op = ~150ns dispatch + free_elems cycles; fp16 unit-stride runs 2x
    (STT runs 1x).  Broadcasts on outer dims keep 2x.
  - The ACT (scalar) queue is a serial second critical path: anything the
    serial chain needs (q for the first quads, b pos-0) must NOT ride it.
    ACT keeps only late-needed work: q for strokes 8+, pass bits, partial
    copies.
  - Each dma_start costs ~700ns issue on its queue engine; alpha goes first
    in small chunks, colors follow per-quad on the same Sync queue
    (concurrent big transfers on another queue starve the alpha DMA).
  - fp16 everywhere passes easily (measured 1.0e-3 vs 2e-2 tolerance).
  - Depth 20 suffices for this input (every pixel's 10th passing stroke is
    within the top 20; checked on host, exact numpy fallback otherwise).
  - PE accumulates ta*c into PSUM via fp16 identity matmuls.

Sharding: pure data parallel, one batch element per NeuronCore (B=8).
"""

import numpy as np

B, N, W, K = 8, 256, 128, 10
ALPHA_THRESH = 0.1
D = 20          # strokes processed from the top (host-verified sufficient)
P = 128         # partitions (pixel rows)
F = 128         # free dim (pixel cols)
NCORES = 8

_nc_cache = {}


def _build_nc(depth):
    import concourse.bass as bass  # noqa: F401
    import concourse.tile as tile
    from concourse import bacc, mybir
    from concourse.vector_clock import ScopedClock

    op = mybir.AluOpType
    act = mybir.ActivationFunctionType
    f16 = mybir.dt.float16
    f32 = mybir.dt.float32

    assert depth % 4 == 0
    NQ = depth // 4          # quads (5)
    NG = NQ - 2              # gated quads (strokes 8..depth-1)
    S2 = depth - 8           # strokes in the ACT-side alpha chunk

    class _OneShotTileContext(tile.TileContext):
        """TileContext with a slim exit: the drain alone (it waits on the
        global clock, including output-DMA completion) -- no all-engine
        barriers and no per-semaphore clears."""

        def _drain_and_barrier(self, tick_clock, wait_clock):
            drain_inst = self.nc.sync.drain()
            wait_clock.add_sem_waits(
                drain_inst.ins, ScopedClock({None: tick_clock.global_clock})
            )
            popped = self.nc._tile_sem_poison_stack.pop()
            assert popped is self._sem_poison

    nc = bacc.Bacc("TRN2", target_bir_lowering=False, debug=False)

    ae_d = nc.dram_tensor("ae_in", [P, depth * F], f16, kind="ExternalInput").ap()
    c_d = nc.dram_tensor("color_in", [P, depth * 3 * F], f16, kind="ExternalInput").ap()
    ident_d = nc.dram_tensor("ident_in", [P, P], f16, kind="ExternalInput").ap()
    out_d = nc.dram_tensor("out", [P, 3 * F], f16, kind="ExternalOutput").ap()

    with _OneShotTileContext(nc) as tc:
        with (
            tc.tile_pool(name="const", bufs=1) as constp,
            tc.tile_pool(name="slab", bufs=1) as slabp,
            tc.tile_pool(name="work", bufs=2) as workp,
            tc.tile_pool(name="prodp", bufs=2) as prodp,
            tc.tile_pool(name="psum", bufs=1, space="PSUM") as psump,
        ):
            # ident via SWDGE (gpsimd queue) so it doesn't delay the HWDGE
            # input stream; it's only needed by the first matmul.
            ident = constp.tile([P, P], f16)
            nc.gpsimd.dma_start(ident[:], ident_d)

            T = constp.tile([P, F], f16)
            R = constp.tile([P, F], f16)
            nc.gpsimd.memset(T[:], 1.0)
            part = slabp.tile([P, NG, 4, F], f16)
            nc.gpsimd.memset(part[:, :, 0], 0.0)
            ps1 = slabp.tile([P, 4, F], f16)
            qs1 = slabp.tile([P, 2, F], f16)

            # ACT warmup: trigger activation-table load while input DMA runs
            warm = constp.tile([P, 8], f16)
            nc.gpsimd.memset(warm[:], 0.5)
            wout = constp.tile([P, 8], f16)
            nc.scalar.sign(wout[:], warm[:])
            nc.scalar.activation(wout[:], warm[:], act.Relu, bias=1.0, scale=-1.0)

            # ---- input DMAs: alpha in (4, 4, 12)-stroke chunks first, then
            # per-quad colors, all on the Sync queue.
            ae1 = slabp.tile([P, 8, F], f16)
            ae2 = slabp.tile([P, S2, F], f16)
            nc.sync.dma_start(
                ae1[:, 0:4], ae_d[:, : 4 * F].rearrange("p (s f) -> p s f", s=4)
            )
            nc.sync.dma_start(
                ae1[:, 4:8],
                ae_d[:, 4 * F : 8 * F].rearrange("p (s f) -> p s f", s=4),
            )
            nc.sync.dma_start(
                ae2[:], ae_d[:, 8 * F :].rearrange("p (s f) -> p s f", s=S2)
            )
            ctile = slabp.tile([P, depth, 3, F], f16)
            for qi in range(NQ):
                lo = qi * 4 * 3 * F
                nc.sync.dma_start(
                    ctile[:, 4 * qi : 4 * qi + 4],
                    c_d[:, lo : lo + 4 * 3 * F].rearrange(
                        "p (s c f) -> p s c f", s=4, c=3
                    ),
                )

            # ---- serial chain body (emitted interleaved with the precompute
            # so quad 0 runs while later inputs are still in flight) ----
            cacc = psump.tile([P, 3 * F], f32)
            tail = constp.tile([P, 3, F], f16)

            def serial_quad(qi):
                gated = qi >= 2
                bQd = (b1 if qi < 2 else b2t)
                lq = qi if qi < 2 else qi - 2
                b_quad = bQd[:, 4 * lq : 4 * lq + 4]
                T_b = T[:].unsqueeze(1).broadcast_to((P, 4, F))
                T_b2 = T[:].unsqueeze(1).broadcast_to((P, 2, F))
                ta = workp.tile([P, 4, F], f16, tag="ta")
                if not gated:
                    nc.vector.tensor_tensor(ta[:], b_quad, T_b, op=op.mult)
                    # T-update is independent of prod: emit it here so DVE
                    # works through it while this quad's color DMA lands
                    nc.vector.tensor_tensor(T[:], T[:], qq1[:, qi], op=op.mult)
                elif qi == 2:
                    # strokes 8,9 can't be gated (R <= 9): plain; gate 10,11
                    nc.vector.tensor_tensor(
                        ta[:, 0:2], b_quad[:, 0:2], T_b2, op=op.mult
                    )
                    tmp = workp.tile([P, 2, F], f16, tag="tmp")
                    R_b2 = R[:].unsqueeze(1).broadcast_to((P, 2, F))
                    nc.vector.tensor_tensor(
                        tmp[:], part[:, 0, 2:4], R_b2, op=op.add
                    )
                    mb = workp.tile([P, 2, F], f16, tag="mb")
                    nc.vector.scalar_tensor_tensor(
                        mb[:], tmp[:], float(K) - 0.5, b_quad[:, 2:4],
                        op0=op.is_lt, op1=op.mult,
                    )
                    nc.vector.tensor_tensor(ta[:, 2:4], mb[:], T_b2, op=op.mult)
                else:
                    tmp = workp.tile([P, 4, F], f16, tag="tmp4")
                    R_b = R[:].unsqueeze(1).broadcast_to((P, 4, F))
                    nc.vector.tensor_tensor(tmp[:], part[:, lq], R_b, op=op.add)
                    mb = workp.tile([P, 4, F], f16, tag="mb4")
                    nc.vector.scalar_tensor_tensor(
                        mb[:], tmp[:], float(K) - 0.5, b_quad,
                        op0=op.is_lt, op1=op.mult,
                    )
                    nc.vector.tensor_tensor(ta[:], mb[:], T_b, op=op.mult)

                if qi == 1:
                    # fill the wait for quad-1's color DMA with the chunk-1
                    # count ops (pass1 is ready: it rides first on ACT)
                    p1P = pass1[:].rearrange("p (pr two) f -> p pr two f", two=2)
                    nc.vector.tensor_tensor(
                        ps1[:], p1P[:, :, 0], p1P[:, :, 1], op=op.add
                    )
                    ps1P = ps1[:].rearrange("p (qd two) f -> p qd two f", two=2)
                    nc.vector.tensor_tensor(
                        qs1[:], ps1P[:, :, 0], ps1P[:, :, 1], op=op.add
                    )
                    nc.vector.tensor_tensor(
                        R[:], qs1[:, 0], qs1[:, 1], op=op.add
                    )

                # T/R updates only read ta/qs2 (not prod): for the middle
                # gated quads emit them BEFORE prod so DVE works through them
                # during any color-DMA / matmul-sem wait.  The last quad keeps
                # prod first (its prod feeds the tail chain).
                early_tr = gated and qi < NQ - 1
                if early_tr:
                    h = workp.tile([P, 2, F], f16, tag="h")
                    nc.vector.tensor_tensor(
                        h[:], ta[:, 0:2], ta[:, 2:4], op=op.add
                    )
                    nc.vector.tensor_tensor(T[:], T[:], h[:, 0], op=op.subtract)
                    nc.vector.tensor_tensor(T[:], T[:], h[:, 1], op=op.subtract)
                    nc.vector.tensor_tensor(
                        R[:], R[:], qs2[:, lq], op=op.add
                    )

                prod = prodp.tile([P, 4, 3, F], f16, tag="prod")
                ta_b = ta[:].unsqueeze(2).broadcast_to((P, 4, 3, F))
                nc.vector.tensor_tensor(
                    prod[:], ctile[:, 4 * qi : 4 * qi + 4], ta_b, op=op.mult
                )
                # last quad: only 2 strokes via PE (PSUM group closes early so
                # PE drains in the shadow); strokes depth-2, depth-1 summed on
                # DVE into `tail`.
                nmm = 2 if qi == NQ - 1 else 4
                for j in range(nmm):
                    s = 4 * qi + j
                    nc.tensor.matmul(
                        cacc[:], ident[:],
                        prod[:, j].rearrange("p c f -> p (c f)"),
                        start=(s == 0), stop=(s == 4 * (NQ - 1) + 1),
                    )
                if qi == NQ - 1:
                    nc.vector.tensor_tensor(
                        tail[:], prod[:, 2], prod[:, 3], op=op.add
                    )

                # last quad's T update (others already emitted above; the
                # qi==1 R init lives in the gate-machinery section)
                if gated and not early_tr:
                    h = workp.tile([P, 2, F], f16, tag="h")
                    nc.vector.tensor_tensor(
                        h[:], ta[:, 0:2], ta[:, 2:4], op=op.add
                    )
                    nc.vector.tensor_tensor(T[:], T[:], h[:, 0], op=op.subtract)
                    nc.vector.tensor_tensor(T[:], T[:], h[:, 1], op=op.subtract)

            # ---- q = 1 - ae: quads 0,1 on DVE (the serial chain must not
            # wait on the ACT queue); strokes 8+ on ACT.
            q1 = slabp.tile([P, 8, F], f16)
            nc.vector.tensor_scalar(
                q1[:, 0:4], ae1[:, 0:4], -1.0, 1.0, op0=op.mult, op1=op.add
            )
            # E tiles: per-quad exclusive prefix products of q
            # (pos0 = 1 via early memset, pos1 = q0, pos2 = q0q1, pos3 = q0q1q2)
            # so all b-terms come from ONE 2x multiply b = ae * E per chunk.
            E1 = slabp.tile([P, 8, F], f16)
            E2 = slabp.tile([P, S2, F], f16)
            EQ1 = E1[:].rearrange("p (qd s) f -> p qd s f", s=4)
            EQ2 = E2[:].rearrange("p (qd s) f -> p qd s f", s=4)
            nc.gpsimd.memset(EQ1[:, :, 0], 1.0)
            nc.gpsimd.memset(EQ2[:, :, 0], 1.0)
            qQ1 = q1[:].rearrange("p (qd s) f -> p qd s f", s=4)

            # ACT queue order: pass1 first (it's ready as soon as the first
            # two alpha chunks land, and the DVE fills its c1-DMA wait with
            # the pass1-derived count ops), then q2 (alpha-DMA-bound anyway),
            # then the late gate inputs.  (E pos-1 copies for chunk 1 are on
            # DVE: an ACT copy emitted before the DVE q1 write would
            # read-before-write.)
            pass1 = slabp.tile([P, 8, F], f16)
            pass2 = slabp.tile([P, S2 - 1, F], f16)
            nc.scalar.sign(pass1[:], ae1[:])
            q2 = slabp.tile([P, S2, F], f16)
            nc.scalar.activation(q2[:], ae2[:], act.Relu, bias=1.0, scale=-1.0)
            qQ2 = q2[:].rearrange("p (qd s) f -> p qd s f", s=4)
            nc.scalar.copy(EQ2[:, :, 1], qQ2[:, :, 0])
            nc.scalar.sign(pass2[:], ae2[:, : S2 - 1])

            # ---- DVE side of the E / b precompute.  Quad 0's closure only
            # needs the first 4-stroke alpha chunk, so its serial work fills
            # the wait for the second chunk's DMA.
            b1 = slabp.tile([P, 8, F], f16)
            b2t = slabp.tile([P, S2, F], f16)
            qq1 = slabp.tile([P, 2, F], f16)
            q12od = workp.tile([P, 2, F], f16, tag="q12od")
            for lq in range(2):
                sl = slice(lq, lq + 1)
                if lq == 1:  # q for quad 1 (waits on the 2nd alpha chunk)
                    nc.vector.tensor_scalar(
                        q1[:, 4:8], ae1[:, 4:8], -1.0, 1.0,
                        op0=op.mult, op1=op.add,
                    )
                nc.vector.tensor_scalar(
                    EQ1[:, sl, 1], qQ1[:, sl, 0], 1.0, None, op0=op.mult
                )
                nc.vector.tensor_tensor(
                    EQ1[:, sl, 2], qQ1[:, sl, 0], qQ1[:, sl, 1], op=op.mult
                )
                nc.vector.tensor_tensor(
                    q12od[:, sl], qQ1[:, sl, 2], qQ1[:, sl, 3], op=op.mult
                )
                nc.vector.tensor_tensor(
                    qq1[:, sl], EQ1[:, sl, 2], q12od[:, sl], op=op.mult
                )
                nc.vector.tensor_tensor(
                    EQ1[:, sl, 3], EQ1[:, sl, 2], qQ1[:, sl, 2], op=op.mult
                )
                nc.vector.tensor_tensor(
                    b1[:, 4 * lq : 4 * lq + 4],
                    ae1[:, 4 * lq : 4 * lq + 4],
                    E1[:, 4 * lq : 4 * lq + 4], op=op.mult,
                )
                if lq == 0:
                    serial_quad(0)
            serial_quad(1)
            # chunk 2 (quads 2..)
            nc.vector.tensor_tensor(
                EQ2[:, :, 2], qQ2[:, :, 0], qQ2[:, :, 1], op=op.mult
            )
            nc.vector.tensor_tensor(
                EQ2[:, :, 3], EQ2[:, :, 2], qQ2[:, :, 2], op=op.mult
            )
            nc.vector.tensor_tensor(b2t[:], ae2[:], E2[:], op=op.mult)

            # ---- gate machinery (batched): pair/quad sums of pass bits,
            # intra-quad partial prefixes for gated quads.
            npair2 = (S2 - 2) // 2
            ps2 = slabp.tile([P, npair2, F], f16)
            p2P = pass2[:, : 2 * npair2].rearrange(
                "p (pr two) f -> p pr two f", two=2
            )
            nc.vector.tensor_tensor(ps2[:], p2P[:, :, 0], p2P[:, :, 1], op=op.add)

            qs2 = slabp.tile([P, NG - 1, F], f16)
            ps2P = ps2[:, : 2 * (NG - 1)].rearrange(
                "p (qd two) f -> p qd two f", two=2
            )
            nc.vector.tensor_tensor(qs2[:], ps2P[:, :, 0], ps2P[:, :, 1], op=op.add)

            # partials: j=0: 0; j=1: p0; j=2: p0+p1; j=3: p0+p1+p2 per quad
            nc.scalar.copy(part[:, :, 1], pass2[:, 0::4])     # strokes 8,12,16
            nc.scalar.copy(part[:, :, 2], ps2[:, 0::2])       # pairs 4,6,8
            nc.vector.tensor_tensor(
                part[:, :, 3], ps2[:, 0::2], pass2[:, 2::4], op=op.add
            )

            # ---- gated serial quads ----
            for qi in range(2, NQ):
                serial_quad(qi)

            # out = cacc + (tail + T): the tail+T fold runs while PE still
            # drains; only the last adds depend on the final PSUM state.  Two
            # halves so the first DMA's issue overlaps the second add.
            nc.vector.tensor_tensor(
                tail[:], tail[:],
                T[:].unsqueeze(1).broadcast_to((P, 3, F)), op=op.add,
            )
            outt = constp.tile([P, 3, F], f16)
            caccv = cacc[:].rearrange("p (c f) -> p c f", c=3)
            nc.vector.tensor_tensor(
                outt[:, 0:2], caccv[:, 0:2], tail[:, 0:2], op=op.add
            )
            nc.sync.dma_start(
                out_d[:, : 2 * F], outt[:, 0:2].rearrange("p c f -> p (c f)")
            )
            nc.vector.tensor_tensor(
                outt[:, 2:3], caccv[:, 2:3], tail[:, 2:3], op=op.add
            )
            # second half rides the idle GpSimd queue so its ~0.6us issue
            # overlaps the first half's issue on Sync
            nc.gpsimd.dma_start(
                out_d[:, 2 * F :], outt[:, 2:3].rearrange("p c f -> p (c f)")
            )

    nc.compile()
    return nc


def _prep_inputs(color_stroke, alpha, depth):
    """Slice the top `depth` strokes (reversed so stroke 0 = highest index),
    mask alpha by the fp32-exact threshold on host, convert to fp16 and lay
    out per core: ae [P, depth*F], color [P, depth*3*F]."""
    a_r = alpha[:, N - depth :, 0][:, ::-1]          # (B, depth, P, F)
    ae_host = (a_r * (a_r > ALPHA_THRESH)).astype(np.float16)
    c_r = color_stroke[:, N - depth :][:, ::-1].astype(np.float16)  # (B,depth,3,P,F)
    ident = np.eye(P, dtype=np.float16)
    in_maps = []
    for b in range(B):
        ae_core = np.ascontiguousarray(ae_host[b].transpose(1, 0, 2)).reshape(
            P, depth * F
        )
        c_core = np.ascontiguousarray(c_r[b].transpose(2, 0, 1, 3)).reshape(
            P, depth * 3 * F
        )
        in_maps.append({"ae_in": ae_core, "color_in": c_core, "ident_in": ident})
    return in_maps


def _reference_numpy(color_stroke, alpha):
    """Exact replication of the oracle (incl. top-k tie-breaking) on host.
    Only used when the depth-cutoff precondition fails (pathological inputs)."""
    stroke_ids = np.arange(1, N + 1, dtype=np.int32).reshape(1, N, 1, 1)
    draw = stroke_ids * (alpha[:, :, 0] > ALPHA_THRESH).astype(np.int32)  # (B,N,W,W)
    draw_t = np.moveaxis(draw, 1, -1)  # (B,W,W,N)
    idx = np.argsort(-draw_t, axis=-1, kind="stable")[..., :K]  # (B,W,W,K)
    idx = np.moveaxis(idx, -1, 1)[:, :, None]  # (B,K,1,W,W)
    alpha_k = np.take_along_axis(alpha, idx, axis=1)  # (B,K,1,W,W)
    color_k = np.take_along_axis(color_stroke, idx, axis=1)  # (B,K,3,W,W)
    canvas = np.ones((B, 3, W, W), dtype=color_stroke.dtype)
    for i in range(K - 1, -1, -1):
        a = alpha_k[:, i]
        canvas = canvas * (1.0 - a) + a * color_k[:, i]
    return canvas


def kernel(color_stroke, alpha):
    color_stroke = np.asarray(color_stroke, dtype=np.float32)
    alpha = np.asarray(alpha, dtype=np.float32)
    assert color_stroke.shape == (B, N, 3, W, W), color_stroke.shape
    assert alpha.shape == (B, N, 1, W, W), alpha.shape

    # Precondition for the depth cutoff: every pixel finds its 10 passing
    # strokes within the top D.
    top_pass = (alpha[:, N - D :, 0] > ALPHA_THRESH).sum(axis=1)
    if top_pass.min() < K:
        return _reference_numpy(color_stroke, alpha)

    from concourse.bass_utils import run_bass_kernel_spmd

    if D not in _nc_cache:
        _nc_cache[D] = _build_nc(D)
    nc = _nc_cache[D]

    in_maps = _prep_inputs(color_stroke, alpha, D)
    res = run_bass_kernel_spmd(nc, in_maps, core_ids=list(range(NCORES)))

    out = np.empty((B, 3, W, W), dtype=np.float32)
    for b in range(B):
        out[b] = (
            res.results[b]["out"].reshape(P, 3, F).transpose(1, 0, 2)
            .astype(np.float32)
        )
    return out
